# revision 10
# baseline (speedup 1.0000x reference)
"""KoLeo loss (view-expanded) on 8 Trainium2 NeuronCores.

Reference math, per view (T=4 views of X [B=8192, D=1024] fp32):
    xn  = x / ||x||                       (row L2 normalize, fp32)
    m_i = max_{j != i} <xn_i, xn_j>       (masked Gram row max)
    dist_i = ||xn_i - xn_{argmax}|| = sqrt(2 - 2 m_i)   (unit rows; the
             reference's +1e-12 eps terms are < 1e-10 relative -> ignored)
    loss = mean_views( -mean_i log(dist_i) ) = -0.5/(T*B) * sum ln(2 - 2 m_i)

Sharding: data-parallel over query rows with symmetry exploitation. Each
of the 8 cores owns B/8=1024 query rows; its input is np.roll'ed by
-c*1024 rows so the (single SPMD) program always sees its queries as rows
0..1023. Because the Gram matrix is symmetric, each core computes only
the column window [0, 5120) in rolled coordinates (its own rows plus half
the ring, rounded up to whole 1024-col panels). Every unordered pair
{r,s} is covered by at least one endpoint's window. Each core produces:
  - row maxes over its window (per query row), and
  - column maxes over its window (max over its 128-row m-blocks,
    partition dim left unreduced),
and the host combines all partial maxes (max is idempotent, so the
overlap region double-counting is harmless), then computes the final
log-mean in float64.

Per-core device pipeline (v2: fully SBUF-resident, no DRAM scratch):
  prep (per view, per 1024-row panel): [128,4,1024] f32 chunks stream
    from DRAM; ScalarE Square+accum_out produces row sums of squares;
    rsqrt = exp(-0.5*ln(n2)) on ScalarE + one fp32 Newton step on VectorE;
    VectorE tensor_scalar scales rows to bf16; TensorE transposes each
    [128,128] block into PSUM (identity moving operand); ScalarE
    evacuates PSUM -> the persistent XnT tile [128, 8(k), 5120] bf16.
  gram (per view, per panel): TensorE accumulates G blocks [128,1024]
    f32 in PSUM from the resident Q^T [128,8,1024] copy (stationary) and
    XnT (moving); VectorE masks the diagonal (panel 0), row-max-reduces
    each block, and max-accumulates the per-panel column-max tile.
  Emission interleaves gram(t, panel p) with prep(t+1, panel p) so the
  Tile scheduler overlaps next-view normalization under current-view
  Gram; a separate Q^T tile confines the cross-view WAR hazard to one
  cheap copy.
"""

import numpy as np

_B = 8192
_T = 4
_D = 1024
_NCORES = 8
_FP8 = False      # compute the Gram in fp8e4m3 (DoubleRow) instead of bf16
_XSCALE = 16.0 if _FP8 else 1.0  # xn pre-scale to dodge fp8 subnormals

_nc_cache = {}


def _cfg(B, T, D, ncores):
    P = 128
    NQ = B // ncores              # query rows per core
    MB = NQ // P                  # m-blocks
    QCW = 1024                    # gram columns per panel
    NQW = -(-(NQ + B // 2) // QCW)  # panels per core (window, rounded up)
    COLS = NQW * QCW              # column window per core
    KC = D // P                   # contraction chunks
    CH = COLS // P                # row chunks normalized per view
    GRP = CH // NQW               # chunks per panel (= rsqrt batch)
    BW = 512                      # gram block width (= one PSUM bank f32)
    NG = COLS // BW               # global blocks in the window
    # blocks per query m-block window: worst in-window offset (BW-P) plus
    # row offset (P-1) plus ahead-coverage (B/2+1), rounded up to blocks
    WB = -(-((BW - P) + (P - 1) + (B // 2 + 1)) // BW)
    assert COLS <= B and NQ <= QCW and D % P == 0 and GRP == 8 and WB == 9
    return P, NQ, MB, QCW, NQW, COLS, KC, CH, GRP, BW, NG, WB


def _patch_act_tables():
    """Force every ACT table load onto natural_log_exp_and_others (which
    contains square+ln+exp+copy+identity) by emptying all other sets in
    the list handed to bacc's chooser. Positions are preserved so the
    emitted act_func_set_id still indexes the real act_info.json."""
    import functools

    from concourse import bacc, hw_specs

    if getattr(bacc, "_koleo_act_patch", False):
        return
    orig = hw_specs.get_activation_tables

    @functools.cache
    def patched(arch):
        tabs = orig(arch)
        keep = "natural_log_exp_and_others"
        if keep not in tabs:
            return tabs
        return {n: (fns if n == keep else set()) for n, fns in tabs.items()}

    bacc.get_activation_tables = patched
    bacc._koleo_act_patch = True


def build_nc(
    B=_B,
    T=_T,
    D=_D,
    ncores=_NCORES,
    enable_asserts=False,
    debug=False,
    repeat=1,
    fp8=_FP8,
):
    """repeat>1 re-emits the whole per-core program back-to-back; used only
    by test.py to measure marginal exec time above dispatch noise."""
    import concourse.tile as tile
    from concourse import bacc, mybir

    _patch_act_tables()

    P, NQ, MB, QCW, NQW, COLS, KC, CH, GRP, BW, NG, WB = _cfg(B, T, D, ncores)
    MCOLS = T * MB

    f32 = mybir.dt.float32
    bf16 = mybir.dt.bfloat16
    dcomp = mybir.dt.float8e4 if fp8 else bf16
    XS = 16.0 if fp8 else 1.0
    MASK = -4.0 * XS * XS
    AF = mybir.ActivationFunctionType
    ALU = mybir.AluOpType
    AX = mybir.AxisListType

    nc = bacc.Bacc(
        "TRN2",
        target_bir_lowering=False,
        debug=debug,
        enable_asserts=enable_asserts,
    )

    x = nc.dram_tensor("x", [B, T, D], f32, kind="ExternalInput").ap()
    negdiag = nc.dram_tensor("negdiag", [P, P], f32, kind="ExternalInput").ap()
    identin = nc.dram_tensor("ident", [P, P], bf16, kind="ExternalInput").ap()
    maxes = nc.dram_tensor("maxes", [P, MCOLS], f32, kind="ExternalOutput").ap()
    colmax = nc.dram_tensor(
        "colmax", [T, P, COLS], bf16, kind="ExternalOutput"
    ).ap()

    with tile.TileContext(nc) as tc:
        with (
            tc.tile_pool(name="consts", bufs=1) as consts,
            tc.tile_pool(name="xnt", bufs=1) as xnt_pool,
            tc.tile_pool(name="qt", bufs=2) as qt_pool,
            tc.tile_pool(name="xin", bufs=3) as xin_pool,
            tc.tile_pool(name="sq", bufs=2) as sq_pool,
            tc.tile_pool(name="stats", bufs=2) as stats_pool,
            tc.tile_pool(name="small", bufs=4) as small_pool,
            tc.tile_pool(name="xnb", bufs=2) as xnb_pool,
            tc.tile_pool(name="cm", bufs=2) as cm_pool,
            tc.tile_pool(name="acc", bufs=1) as acc_pool,
            tc.tile_pool(name="tp", bufs=2, space="PSUM") as tp_pool,
            tc.tile_pool(name="ps", bufs=5, space="PSUM") as ps_pool,
        ):
            negd = consts.tile([P, P], f32)
            nc.sync.dma_start(out=negd, in_=negdiag)
            idt = consts.tile([P, P], bf16)
            nc.sync.dma_start(out=idt, in_=identin)

            mbuf = acc_pool.tile([P, MCOLS], f32)
            # persistent transposed-normalized window [d-chunk k][cols]
            xnt = xnt_pool.tile([P, KC, COLS], dcomp)

            stats = {}

            def get_stats(t):
                if t not in stats:
                    n2 = stats_pool.tile([P, CH], f32, name=f"n2_{t}", tag="n2")
                    sc = stats_pool.tile([P, CH], f32, name=f"sc_{t}", tag="sc")
                    stats[t] = (n2, sc)
                return stats[t]

            def emit_prep_panel(tt, p):
                rep, t = divmod(tt, T)
                n2, sc = get_stats(tt)
                halves = []
                for half in range(2):
                    c0 = p * GRP + half * 4
                    xm = xin_pool.tile(
                        [P, 4, D], f32, name=f"xin_{tt}_{c0}", tag="xin"
                    )
                    nc.sync.dma_start(
                        out=xm,
                        in_=x[c0 * P:(c0 + 4) * P, t, :].rearrange(
                            "(c p) d -> p c d", p=P
                        ),
                    )
                    for j in range(4):
                        c = c0 + j
                        sqt = sq_pool.tile([P, D], bf16, name=f"sq_{tt}_{c}", tag="sq")
                        nc.scalar.activation(
                            out=sqt,
                            in_=xm[:, j, :],
                            func=AF.Square,
                            accum_out=n2[:, c:c + 1],
                        )
                    halves.append((c0, xm))

                # rsqrt seed via exp(-0.5 ln(n2)) (same ACT table set as
                # Square), then one fp32 Newton step:
                #   s = s0 * (1.5 - 0.5 * n2 * s0^2)
                gs = slice(p * GRP, (p + 1) * GRP)
                lnv = small_pool.tile([P, GRP], f32, name=f"lnv_{tt}_{p}", tag="lnv")
                nc.scalar.activation(out=lnv, in_=n2[:, gs], func=AF.Ln)
                s0 = small_pool.tile([P, GRP], f32, name=f"s0_{tt}_{p}", tag="s0")
                nc.scalar.activation(out=s0, in_=lnv, func=AF.Exp, scale=-0.5)
                t1 = small_pool.tile([P, GRP], f32, name=f"t1_{tt}_{p}", tag="t1")
                nc.vector.tensor_mul(t1, s0, s0)
                t2 = small_pool.tile([P, GRP], f32, name=f"t2_{tt}_{p}", tag="t2")
                nc.vector.tensor_mul(t2, t1, n2[:, gs])
                t3 = small_pool.tile([P, GRP], f32, name=f"t3_{tt}_{p}", tag="t3")
                nc.vector.tensor_scalar(t3, t2, -0.5, 1.5, ALU.mult, ALU.add)
                nc.vector.tensor_mul(sc[:, gs], s0, t3)

                for c0, xm in halves:
                    for j in range(4):
                        c = c0 + j
                        xnb = xnb_pool.tile(
                            [P, D], bf16, name=f"xnb_{tt}_{c}", tag="xnb"
                        )
                        nc.vector.tensor_scalar_mul(
                            xnb, xm[:, j, :], sc[:, c:c + 1]
                        )
                        tp = tp_pool.tile(
                            [P, KC, P], bf16, name=f"tp_{tt}_{c}", tag="tp"
                        )
                        for k in range(KC):
                            nc.tensor.transpose(
                                tp[:, k, :], xnb[:, k * P:(k + 1) * P], idt
                            )
                        nc.scalar.activation(
                            out=xnt[:, :, c * P:(c + 1) * P],
                            in_=tp,
                            func=AF.Copy,
                            scale=XS,
                        )

            def mi_base(mi):
                return (mi * P) // BW * BW

            def emit_gram_block(tt, g, qt, cm):
                """All (mi, global block g) gram tiles; per-mi windows are
                [mi_base, mi_base + WB*BW) so every unordered pair is
                covered by one side (d<4097 ahead per query row)."""
                rep, t = divmod(tt, T)
                col0 = g * BW
                mis = [mi for mi in range(MB) if 0 <= g - mi_base(mi) // BW < WB]
                for i, mi in enumerate(mis):
                    ps = ps_pool.tile(
                        [P, BW], f32, name=f"ps_{tt}_{g}_{mi}", tag="ps"
                    )
                    if fp8:
                        for kp in range(KC // 2):
                            nc.tensor.matmul(
                                ps,
                                qt[:, 2 * kp:2 * kp + 2, mi * P:(mi + 1) * P],
                                xnt[:, 2 * kp:2 * kp + 2, col0:col0 + BW],
                                start=(kp == 0),
                                stop=(kp == KC // 2 - 1),
                                perf_mode=mybir.MatmulPerfMode.DoubleRow,
                            )
                    else:
                        for k in range(KC):
                            nc.tensor.matmul(
                                ps,
                                qt[:, k, mi * P:(mi + 1) * P],
                                xnt[:, k, col0:col0 + BW],
                                start=(k == 0),
                                stop=(k == KC - 1),
                            )
                    base = mi_base(mi)
                    wfirst = g == base // BW  # first block of mi's window
                    if wfirst:
                        # mask the self-dot: psum diag window += -4*I
                        off = mi * P - base
                        nc.vector.tensor_tensor(
                            ps[:, off:off + P],
                            ps[:, off:off + P],
                            negd,
                            op=ALU.add,
                        )
                    col = t * MB + mi
                    if wfirst:
                        rm = mbuf[:, col:col + 1]
                    else:
                        rm = small_pool.tile(
                            [P, 1], f32, name=f"qm_{tt}_{g}_{mi}", tag="qm"
                        )
                    nc.vector.reduce_max(rm, ps, axis=AX.X)
                    if i == 0:
                        nc.vector.tensor_copy(cm[:, col0:col0 + BW], ps)
                    else:
                        nc.vector.tensor_tensor(
                            cm[:, col0:col0 + BW],
                            cm[:, col0:col0 + BW],
                            ps,
                            op=ALU.max,
                        )
                    if not wfirst:
                        nc.vector.tensor_tensor(
                            mbuf[:, col:col + 1],
                            mbuf[:, col:col + 1],
                            rm,
                            op=ALU.max,
                        )

            # ---- emission schedule: pipeline prep(t+1) under gram(t) ----
            TT = repeat * T
            for p in range(NQW):
                emit_prep_panel(0, p)
            for tt in range(TT):
                t = tt % T
                qt = qt_pool.tile([P, KC, NQ], dcomp, name=f"qt_{tt}", tag="qt")
                nc.vector.tensor_copy(qt, xnt[:, :, 0:NQ])
                cm = cm_pool.tile([P, COLS], bf16, name=f"cm_{tt}", tag="cm")
                for g in range(NG):
                    emit_gram_block(tt, g, qt, cm)
                    if g % 2 == 1 and tt + 1 < TT:
                        emit_prep_panel(tt + 1, (g - 1) // 2)
                nc.sync.dma_start(out=colmax[t, :, :], in_=cm)

            nc.sync.dma_start(out=maxes, in_=mbuf)

    nc.compile()
    return nc


def make_negdiag(maskval=None):
    if maskval is None:
        maskval = -4.0 * _XSCALE * _XSCALE
    return (maskval * np.eye(128)).astype(np.float32)


def make_ident():
    from concourse import mybir

    return np.eye(128).astype(mybir.dt.np(mybir.dt.bfloat16))


def make_in_maps(x, B=_B, T=_T, D=_D, ncores=_NCORES):
    """x: [B, T, D] fp32 full input -> per-core rolled input maps."""
    x = np.ascontiguousarray(x, dtype=np.float32)
    assert x.shape == (B, T, D)
    nd = make_negdiag()
    idt = make_ident()
    NQ = B // ncores
    in_maps = []
    for c in range(ncores):
        xr = np.roll(x, -c * NQ, axis=0) if c else x
        in_maps.append(
            {"x": np.ascontiguousarray(xr), "negdiag": nd, "ident": idt}
        )
    return in_maps


def combine_maxes(results, B=_B, T=_T, D=_D, ncores=_NCORES):
    """Combine per-core row/column max partials -> M [T, B] (fp64)."""
    P, NQ, MB, QCW, NQW, COLS, KC, CH, GRP, BW, NG, WB = _cfg(B, T, D, ncores)
    M = np.full((T, B), -np.inf)
    for c, r in enumerate(results):
        rowmax = np.asarray(r["maxes"], dtype=np.float64)  # [128, T*MB]
        for t in range(T):
            for mi in range(MB):
                rows = (c * NQ + mi * P + np.arange(P)) % B
                M[t, rows] = np.maximum(M[t, rows], rowmax[:, t * MB + mi])
        cmx = np.asarray(r["colmax"], dtype=np.float64)  # [T, 128, COLS]
        cmx = cmx.max(axis=1)  # [T, COLS]
        gcols = (c * NQ + np.arange(COLS)) % B
        for t in range(T):
            np.maximum.at(M[t], gcols, cmx[t])
    return M


def assemble_output(results, B=_B, T=_T, D=_D, ncores=_NCORES):
    M = combine_maxes(results, B, T, D, ncores) / (_XSCALE * _XSCALE)
    loss = -0.5 * np.log(2.0 - 2.0 * M).mean()
    return np.asarray(loss, dtype=np.float32)


def kernel(episodes_vectors: np.ndarray) -> np.ndarray:
    from concourse.bass_utils import run_bass_kernel_spmd

    key = (_B, _T, _D, _NCORES)
    if key not in _nc_cache:
        _nc_cache[key] = build_nc()
    nc = _nc_cache[key]

    in_maps = make_in_maps(episodes_vectors)
    last_err = None
    for _attempt in range(3):
        try:
            res = run_bass_kernel_spmd(nc, in_maps, list(range(_NCORES)))
            return assemble_output(res.results)
        except Exception as e:  # transient PJRT/tunnel INTERNAL errors
            last_err = e
    raise last_err


if __name__ == "__main__":
    inputs = {
        "episodes_vectors": np.random.default_rng(0)
        .standard_normal((_B, _T, _D))
        .astype(np.float32)
    }
    print(kernel(**inputs))


# revision 11
# speedup vs baseline: 1.1354x; 1.1354x over previous
"""KoLeo loss (view-expanded) on 8 Trainium2 NeuronCores.

Reference math, per view (T=4 views of X [B=8192, D=1024] fp32):
    xn  = x / ||x||                       (row L2 normalize, fp32)
    m_i = max_{j != i} <xn_i, xn_j>       (masked Gram row max)
    dist_i = ||xn_i - xn_{argmax}|| = sqrt(2 - 2 m_i)   (unit rows; the
             reference's +1e-12 eps terms are < 1e-10 relative -> ignored)
    loss = mean_views( -mean_i log(dist_i) ) = -0.5/(T*B) * sum ln(2 - 2 m_i)

Sharding: data-parallel over query rows with symmetry exploitation. Each
of the 8 cores owns B/8=1024 query rows; its input is np.roll'ed by
-c*1024 rows so the (single SPMD) program always sees its queries as rows
0..1023. Because the Gram matrix is symmetric, each core computes only
the column window [0, 5120) in rolled coordinates (its own rows plus half
the ring, rounded up to whole 1024-col panels). Every unordered pair
{r,s} is covered by at least one endpoint's window. Each core produces:
  - row maxes over its window (per query row), and
  - column maxes over its window (max over its 128-row m-blocks,
    partition dim left unreduced),
and the host combines all partial maxes (max is idempotent, so the
overlap region double-counting is harmless), then computes the final
log-mean in float64.

Per-core device pipeline (v2: fully SBUF-resident, no DRAM scratch):
  prep (per view, per 1024-row panel): [128,4,1024] f32 chunks stream
    from DRAM; ScalarE Square+accum_out produces row sums of squares;
    rsqrt = exp(-0.5*ln(n2)) on ScalarE + one fp32 Newton step on VectorE;
    VectorE tensor_scalar scales rows to bf16; TensorE transposes each
    [128,128] block into PSUM (identity moving operand); ScalarE
    evacuates PSUM -> the persistent XnT tile [128, 8(k), 5120] bf16.
  gram (per view, per panel): TensorE accumulates G blocks [128,1024]
    f32 in PSUM from the resident Q^T [128,8,1024] copy (stationary) and
    XnT (moving); VectorE masks the diagonal (panel 0), row-max-reduces
    each block, and max-accumulates the per-panel column-max tile.
  Emission interleaves gram(t, panel p) with prep(t+1, panel p) so the
  Tile scheduler overlaps next-view normalization under current-view
  Gram; a separate Q^T tile confines the cross-view WAR hazard to one
  cheap copy.
"""

import numpy as np

_B = 8192
_T = 4
_D = 1024
_NCORES = 8
_FP8 = False      # compute the Gram in fp8e4m3 (DoubleRow) instead of bf16
_XSCALE = 16.0 if _FP8 else 1.0  # xn pre-scale to dodge fp8 subnormals

_nc_cache = {}


def _cfg(B, T, D, ncores):
    P = 128
    NQ = B // ncores              # query rows per core
    MB = NQ // P                  # m-blocks
    QCW = 1024                    # gram columns per panel
    NQW = -(-(NQ + B // 2) // QCW)  # panels per core (window, rounded up)
    COLS = NQW * QCW              # column window per core
    KC = D // P                   # contraction chunks
    CH = COLS // P                # row chunks normalized per view
    GRP = CH // NQW               # chunks per panel (= rsqrt batch)
    BW = 512                      # gram block width (= one PSUM bank f32)
    NG = COLS // BW               # global blocks in the window
    # blocks per query m-block window: worst in-window offset (BW-P) plus
    # row offset (P-1) plus ahead-coverage (B/2+1), rounded up to blocks
    WB = -(-((BW - P) + (P - 1) + (B // 2 + 1)) // BW)
    assert COLS <= B and NQ <= QCW and D % P == 0 and GRP == 8 and WB == 9
    return P, NQ, MB, QCW, NQW, COLS, KC, CH, GRP, BW, NG, WB


def _patch_act_tables():
    """Force every ACT table load onto natural_log_exp_and_others (which
    contains square+ln+exp+copy+identity) by emptying all other sets in
    the list handed to bacc's chooser. Positions are preserved so the
    emitted act_func_set_id still indexes the real act_info.json."""
    import functools

    from concourse import bacc, hw_specs

    if getattr(bacc, "_koleo_act_patch", False):
        return
    orig = hw_specs.get_activation_tables

    @functools.cache
    def patched(arch):
        tabs = orig(arch)
        keep = "natural_log_exp_and_others"
        if keep not in tabs:
            return tabs
        return {n: (fns if n == keep else set()) for n, fns in tabs.items()}

    bacc.get_activation_tables = patched
    bacc._koleo_act_patch = True


def build_nc(
    B=_B,
    T=_T,
    D=_D,
    ncores=_NCORES,
    enable_asserts=False,
    debug=False,
    repeat=1,
    fp8=_FP8,
):
    """repeat>1 re-emits the whole per-core program back-to-back; used only
    by test.py to measure marginal exec time above dispatch noise."""
    import concourse.tile as tile
    from concourse import bacc, mybir

    _patch_act_tables()

    P, NQ, MB, QCW, NQW, COLS, KC, CH, GRP, BW, NG, WB = _cfg(B, T, D, ncores)
    MCOLS = T * MB

    f32 = mybir.dt.float32
    bf16 = mybir.dt.bfloat16
    dcomp = mybir.dt.float8e4 if fp8 else bf16
    XS = 16.0 if fp8 else 1.0
    MASK = -4.0 * XS * XS
    AF = mybir.ActivationFunctionType
    ALU = mybir.AluOpType
    AX = mybir.AxisListType

    nc = bacc.Bacc(
        "TRN2",
        target_bir_lowering=False,
        debug=debug,
        enable_asserts=enable_asserts,
    )

    x = nc.dram_tensor("x", [B, T, D], f32, kind="ExternalInput").ap()
    negdiag = nc.dram_tensor("negdiag", [P, P], f32, kind="ExternalInput").ap()
    identin = nc.dram_tensor("ident", [P, P], bf16, kind="ExternalInput").ap()
    maxes = nc.dram_tensor("maxes", [P, MCOLS], f32, kind="ExternalOutput").ap()
    colmax = nc.dram_tensor(
        "colmax", [T, P, COLS], bf16, kind="ExternalOutput"
    ).ap()

    with tile.TileContext(nc) as tc:
        with (
            tc.tile_pool(name="consts", bufs=1) as consts,
            tc.tile_pool(name="xnt", bufs=1) as xnt_pool,
            tc.tile_pool(name="qt", bufs=2) as qt_pool,
            tc.tile_pool(name="xin", bufs=3) as xin_pool,
            tc.tile_pool(name="sq", bufs=2) as sq_pool,
            tc.tile_pool(name="stats", bufs=2) as stats_pool,
            tc.tile_pool(name="small", bufs=4) as small_pool,
            tc.tile_pool(name="xnb", bufs=2) as xnb_pool,
            tc.tile_pool(name="cm", bufs=2) as cm_pool,
            tc.tile_pool(name="acc", bufs=1) as acc_pool,
            tc.tile_pool(name="tp", bufs=2, space="PSUM") as tp_pool,
            tc.tile_pool(name="ps", bufs=3, space="PSUM") as ps_pool,
        ):
            negd = consts.tile([P, P], f32)
            nc.sync.dma_start(out=negd, in_=negdiag)
            idt = consts.tile([P, P], bf16)
            nc.sync.dma_start(out=idt, in_=identin)

            mbuf = acc_pool.tile([P, MCOLS], f32)
            # persistent transposed-normalized window [d-chunk k][cols]
            xnt = xnt_pool.tile([P, KC, COLS], dcomp)

            stats = {}

            def get_stats(t):
                if t not in stats:
                    n2 = stats_pool.tile([P, CH], f32, name=f"n2_{t}", tag="n2")
                    sc = stats_pool.tile([P, CH], f32, name=f"sc_{t}", tag="sc")
                    stats[t] = (n2, sc)
                return stats[t]

            def emit_prep_panel(tt, p):
                rep, t = divmod(tt, T)
                n2, sc = get_stats(tt)
                halves = []
                for half in range(2):
                    c0 = p * GRP + half * 4
                    xm = xin_pool.tile(
                        [P, 4, D], f32, name=f"xin_{tt}_{c0}", tag="xin"
                    )
                    nc.sync.dma_start(
                        out=xm,
                        in_=x[c0 * P:(c0 + 4) * P, t, :].rearrange(
                            "(c p) d -> p c d", p=P
                        ),
                    )
                    for j in range(4):
                        c = c0 + j
                        sqt = sq_pool.tile([P, D], bf16, name=f"sq_{tt}_{c}", tag="sq")
                        nc.scalar.activation(
                            out=sqt,
                            in_=xm[:, j, :],
                            func=AF.Square,
                            accum_out=n2[:, c:c + 1],
                        )
                    halves.append((c0, xm))

                # rsqrt seed via exp(-0.5 ln(n2)) (same ACT table set as
                # Square), then one fp32 Newton step:
                #   s = s0 * (1.5 - 0.5 * n2 * s0^2)
                gs = slice(p * GRP, (p + 1) * GRP)
                lnv = small_pool.tile([P, GRP], f32, name=f"lnv_{tt}_{p}", tag="lnv")
                nc.scalar.activation(out=lnv, in_=n2[:, gs], func=AF.Ln)
                s0 = small_pool.tile([P, GRP], f32, name=f"s0_{tt}_{p}", tag="s0")
                nc.scalar.activation(out=s0, in_=lnv, func=AF.Exp, scale=-0.5)
                t1 = small_pool.tile([P, GRP], f32, name=f"t1_{tt}_{p}", tag="t1")
                nc.vector.tensor_mul(t1, s0, s0)
                t2 = small_pool.tile([P, GRP], f32, name=f"t2_{tt}_{p}", tag="t2")
                nc.vector.tensor_mul(t2, t1, n2[:, gs])
                t3 = small_pool.tile([P, GRP], f32, name=f"t3_{tt}_{p}", tag="t3")
                nc.vector.tensor_scalar(t3, t2, -0.5, 1.5, ALU.mult, ALU.add)
                nc.vector.tensor_mul(sc[:, gs], s0, t3)

                for c0, xm in halves:
                    for j in range(4):
                        c = c0 + j
                        xnb = xnb_pool.tile(
                            [P, D], bf16, name=f"xnb_{tt}_{c}", tag="xnb"
                        )
                        nc.vector.tensor_scalar_mul(
                            xnb, xm[:, j, :], sc[:, c:c + 1]
                        )
                        tp = tp_pool.tile(
                            [P, KC, P], bf16, name=f"tp_{tt}_{c}", tag="tp"
                        )
                        for k in range(KC):
                            nc.tensor.transpose(
                                tp[:, k, :], xnb[:, k * P:(k + 1) * P], idt
                            )
                        nc.scalar.activation(
                            out=xnt[:, :, c * P:(c + 1) * P],
                            in_=tp,
                            func=AF.Copy,
                            scale=XS,
                        )

            def mi_base(mi):
                return (mi * P) // BW * BW

            def emit_gram_pg(tt, pg, qt, cm):
                """All (mi, pair-group pg) gram tiles. Per-mi windows are
                [mi_base, mi_base + WB*BW) so every unordered pair is
                covered by one side (d<4097 ahead per query row). Blocks
                are processed in QCW-wide pair-groups so one LDWEIGHTS
                serves two 512-col matmuls and DVE ops run 1024 wide."""
                rep, t = divmod(tt, T)
                # widest-window mi first: its tensor_copy initializes the
                # cm range that the narrower mis then max into
                mis = sorted(
                    range(MB),
                    key=lambda mi: -(
                        min(mi_base(mi) + WB * BW, (pg + 1) * QCW)
                        - max(mi_base(mi), pg * QCW)
                    ),
                )
                for i, mi in enumerate(mis):
                    base = mi_base(mi)
                    lo = max(base, pg * QCW)
                    hi = min(base + WB * BW, (pg + 1) * QCW)
                    W = hi - lo
                    assert W in (BW, QCW)
                    ps = ps_pool.tile(
                        [P, QCW], f32, name=f"ps_{tt}_{pg}_{mi}", tag="ps"
                    )
                    for nb in range(W // BW):
                        c0, c1 = lo + nb * BW, lo + (nb + 1) * BW
                        if fp8:
                            for kp in range(KC // 2):
                                nc.tensor.matmul(
                                    ps[:, nb * BW:(nb + 1) * BW],
                                    qt[:, 2 * kp:2 * kp + 2, mi * P:(mi + 1) * P],
                                    xnt[:, 2 * kp:2 * kp + 2, c0:c1],
                                    start=(kp == 0),
                                    stop=(kp == KC // 2 - 1),
                                    perf_mode=mybir.MatmulPerfMode.DoubleRow,
                                )
                        else:
                            for k in range(KC):
                                nc.tensor.matmul(
                                    ps[:, nb * BW:(nb + 1) * BW],
                                    qt[:, k, mi * P:(mi + 1) * P],
                                    xnt[:, k, c0:c1],
                                    start=(k == 0),
                                    stop=(k == KC - 1),
                                )
                    psw = ps[:, 0:W]
                    if pg == 0:
                        # mask the self-dot: psum diag window += -4*I
                        off = mi * P - lo
                        nc.vector.tensor_tensor(
                            ps[:, off:off + P],
                            ps[:, off:off + P],
                            negd,
                            op=ALU.add,
                        )
                    col = t * MB + mi
                    if pg == 0:
                        rm = mbuf[:, col:col + 1]
                    else:
                        rm = small_pool.tile(
                            [P, 1], f32, name=f"qm_{tt}_{pg}_{mi}", tag="qm"
                        )
                    nc.vector.reduce_max(rm, psw, axis=AX.X)
                    if i == 0:
                        nc.vector.tensor_copy(cm[:, lo:hi], psw)
                    else:
                        nc.vector.tensor_tensor(
                            cm[:, lo:hi],
                            cm[:, lo:hi],
                            psw,
                            op=ALU.max,
                        )
                    if pg != 0:
                        nc.vector.tensor_tensor(
                            mbuf[:, col:col + 1],
                            mbuf[:, col:col + 1],
                            rm,
                            op=ALU.max,
                        )

            # ---- emission schedule: pipeline prep(t+1) under gram(t) ----
            TT = repeat * T
            for p in range(NQW):
                emit_prep_panel(0, p)
            for tt in range(TT):
                t = tt % T
                qt = qt_pool.tile([P, KC, NQ], dcomp, name=f"qt_{tt}", tag="qt")
                nc.vector.tensor_copy(qt, xnt[:, :, 0:NQ])
                cm = cm_pool.tile([P, COLS], bf16, name=f"cm_{tt}", tag="cm")
                for pg in range(NQW):
                    emit_gram_pg(tt, pg, qt, cm)
                    if tt + 1 < TT:
                        emit_prep_panel(tt + 1, pg)
                nc.sync.dma_start(out=colmax[t, :, :], in_=cm)

            nc.sync.dma_start(out=maxes, in_=mbuf)

    nc.compile()
    return nc


def make_negdiag(maskval=None):
    if maskval is None:
        maskval = -4.0 * _XSCALE * _XSCALE
    return (maskval * np.eye(128)).astype(np.float32)


def make_ident():
    from concourse import mybir

    return np.eye(128).astype(mybir.dt.np(mybir.dt.bfloat16))


def make_in_maps(x, B=_B, T=_T, D=_D, ncores=_NCORES):
    """x: [B, T, D] fp32 full input -> per-core rolled input maps."""
    x = np.ascontiguousarray(x, dtype=np.float32)
    assert x.shape == (B, T, D)
    nd = make_negdiag()
    idt = make_ident()
    NQ = B // ncores
    in_maps = []
    for c in range(ncores):
        xr = np.roll(x, -c * NQ, axis=0) if c else x
        in_maps.append(
            {"x": np.ascontiguousarray(xr), "negdiag": nd, "ident": idt}
        )
    return in_maps


def combine_maxes(results, B=_B, T=_T, D=_D, ncores=_NCORES):
    """Combine per-core row/column max partials -> M [T, B] (fp64)."""
    P, NQ, MB, QCW, NQW, COLS, KC, CH, GRP, BW, NG, WB = _cfg(B, T, D, ncores)
    M = np.full((T, B), -np.inf)
    for c, r in enumerate(results):
        rowmax = np.asarray(r["maxes"], dtype=np.float64)  # [128, T*MB]
        for t in range(T):
            for mi in range(MB):
                rows = (c * NQ + mi * P + np.arange(P)) % B
                M[t, rows] = np.maximum(M[t, rows], rowmax[:, t * MB + mi])
        cmx = np.asarray(r["colmax"], dtype=np.float64)  # [T, 128, COLS]
        cmx = cmx.max(axis=1)  # [T, COLS]
        gcols = (c * NQ + np.arange(COLS)) % B
        for t in range(T):
            np.maximum.at(M[t], gcols, cmx[t])
    return M


def assemble_output(results, B=_B, T=_T, D=_D, ncores=_NCORES):
    M = combine_maxes(results, B, T, D, ncores) / (_XSCALE * _XSCALE)
    loss = -0.5 * np.log(2.0 - 2.0 * M).mean()
    return np.asarray(loss, dtype=np.float32)


def kernel(episodes_vectors: np.ndarray) -> np.ndarray:
    from concourse.bass_utils import run_bass_kernel_spmd

    key = (_B, _T, _D, _NCORES)
    if key not in _nc_cache:
        _nc_cache[key] = build_nc()
    nc = _nc_cache[key]

    in_maps = make_in_maps(episodes_vectors)
    last_err = None
    for _attempt in range(3):
        try:
            res = run_bass_kernel_spmd(nc, in_maps, list(range(_NCORES)))
            return assemble_output(res.results)
        except Exception as e:  # transient PJRT/tunnel INTERNAL errors
            last_err = e
    raise last_err


if __name__ == "__main__":
    inputs = {
        "episodes_vectors": np.random.default_rng(0)
        .standard_normal((_B, _T, _D))
        .astype(np.float32)
    }
    print(kernel(**inputs))


# revision 12
# speedup vs baseline: 1.4992x; 1.3204x over previous
"""KoLeo loss (view-expanded) on 8 Trainium2 NeuronCores.

Reference math, per view (T=4 views of X [B=8192, D=1024] fp32):
    xn  = x / ||x||                       (row L2 normalize, fp32)
    m_i = max_{j != i} <xn_i, xn_j>       (masked Gram row max)
    dist_i = ||xn_i - xn_{argmax}|| = sqrt(2 - 2 m_i)   (unit rows; the
             reference's +1e-12 eps terms are < 1e-10 relative -> ignored)
    loss = mean_views( -mean_i log(dist_i) ) = -0.5/(T*B) * sum ln(2 - 2 m_i)

Sharding: data-parallel over query rows with symmetry exploitation. Each
of the 8 cores owns B/8=1024 query rows; its input is np.roll'ed by
-c*1024 rows so the (single SPMD) program always sees its queries as rows
0..1023. Because the Gram matrix is symmetric, each core computes only
the column window [0, 5120) in rolled coordinates (its own rows plus half
the ring, rounded up to whole 1024-col panels). Every unordered pair
{r,s} is covered by at least one endpoint's window. Each core produces:
  - row maxes over its window (per query row), and
  - column maxes over its window (max over its 128-row m-blocks,
    partition dim left unreduced),
and the host combines all partial maxes (max is idempotent, so the
overlap region double-counting is harmless), then computes the final
log-mean in float64.

Per-core device pipeline (v2: fully SBUF-resident, no DRAM scratch):
  prep (per view, per 1024-row panel): [128,4,1024] f32 chunks stream
    from DRAM; ScalarE Square+accum_out produces row sums of squares;
    rsqrt = exp(-0.5*ln(n2)) on ScalarE + one fp32 Newton step on VectorE;
    VectorE tensor_scalar scales rows to bf16; TensorE transposes each
    [128,128] block into PSUM (identity moving operand); ScalarE
    evacuates PSUM -> the persistent XnT tile [128, 8(k), 5120] bf16.
  gram (per view, per panel): TensorE accumulates G blocks [128,1024]
    f32 in PSUM from the resident Q^T [128,8,1024] copy (stationary) and
    XnT (moving); VectorE masks the diagonal (panel 0), row-max-reduces
    each block, and max-accumulates the per-panel column-max tile.
  Emission interleaves gram(t, panel p) with prep(t+1, panel p) so the
  Tile scheduler overlaps next-view normalization under current-view
  Gram; a separate Q^T tile confines the cross-view WAR hazard to one
  cheap copy.
"""

import numpy as np

_B = 8192
_T = 4
_D = 1024
_NCORES = 8
_FP8 = True       # compute the Gram in fp8e4m3 (DoubleRow) instead of bf16
_XSCALE = 16.0 if _FP8 else 1.0  # xn pre-scale to dodge fp8 subnormals

_nc_cache = {}


def _cfg(B, T, D, ncores):
    P = 128
    NQ = B // ncores              # query rows per core
    MB = NQ // P                  # m-blocks
    QCW = 1024                    # gram columns per panel
    NQW = -(-(NQ + B // 2) // QCW)  # panels per core (window, rounded up)
    COLS = NQW * QCW              # column window per core
    KC = D // P                   # contraction chunks
    CH = COLS // P                # row chunks normalized per view
    GRP = CH // NQW               # chunks per panel (= rsqrt batch)
    BW = 512                      # gram block width (= one PSUM bank f32)
    NG = COLS // BW               # global blocks in the window
    # blocks per query m-block window: worst in-window offset (BW-P) plus
    # row offset (P-1) plus ahead-coverage (B/2+1), rounded up to blocks
    WB = -(-((BW - P) + (P - 1) + (B // 2 + 1)) // BW)
    assert COLS <= B and NQ <= QCW and D % P == 0 and GRP == 8 and WB == 9
    return P, NQ, MB, QCW, NQW, COLS, KC, CH, GRP, BW, NG, WB


def _patch_act_tables():
    """Force every ACT table load onto natural_log_exp_and_others (which
    contains square+ln+exp+copy+identity) by emptying all other sets in
    the list handed to bacc's chooser. Positions are preserved so the
    emitted act_func_set_id still indexes the real act_info.json."""
    import functools

    from concourse import bacc, hw_specs

    if getattr(bacc, "_koleo_act_patch", False):
        return
    orig = hw_specs.get_activation_tables

    @functools.cache
    def patched(arch):
        tabs = orig(arch)
        keep = "natural_log_exp_and_others"
        if keep not in tabs:
            return tabs
        return {n: (fns if n == keep else set()) for n, fns in tabs.items()}

    bacc.get_activation_tables = patched
    bacc._koleo_act_patch = True


def build_nc(
    B=_B,
    T=_T,
    D=_D,
    ncores=_NCORES,
    enable_asserts=False,
    debug=False,
    repeat=1,
    fp8=_FP8,
):
    """repeat>1 re-emits the whole per-core program back-to-back; used only
    by test.py to measure marginal exec time above dispatch noise."""
    import concourse.tile as tile
    from concourse import bacc, mybir

    _patch_act_tables()

    P, NQ, MB, QCW, NQW, COLS, KC, CH, GRP, BW, NG, WB = _cfg(B, T, D, ncores)
    MCOLS = T * MB

    f32 = mybir.dt.float32
    bf16 = mybir.dt.bfloat16
    dcomp = mybir.dt.float8e4 if fp8 else bf16
    XS = 16.0 if fp8 else 1.0
    MASK = -4.0 * XS * XS
    AF = mybir.ActivationFunctionType
    ALU = mybir.AluOpType
    AX = mybir.AxisListType

    nc = bacc.Bacc(
        "TRN2",
        target_bir_lowering=False,
        debug=debug,
        enable_asserts=enable_asserts,
    )

    x = nc.dram_tensor("x", [B, T, D], f32, kind="ExternalInput").ap()
    negdiag = nc.dram_tensor("negdiag", [P, P], f32, kind="ExternalInput").ap()
    identin = nc.dram_tensor("ident", [P, P], bf16, kind="ExternalInput").ap()
    maxes = nc.dram_tensor("maxes", [P, MCOLS], f32, kind="ExternalOutput").ap()
    colmax = nc.dram_tensor(
        "colmax", [T, P, COLS], bf16, kind="ExternalOutput"
    ).ap()

    with tile.TileContext(nc) as tc:
        with (
            tc.tile_pool(name="consts", bufs=1) as consts,
            tc.tile_pool(name="xnt", bufs=1) as xnt_pool,
            tc.tile_pool(name="qt", bufs=2) as qt_pool,
            tc.tile_pool(name="xin", bufs=3) as xin_pool,
            tc.tile_pool(name="sq", bufs=2) as sq_pool,
            tc.tile_pool(name="stats", bufs=2) as stats_pool,
            tc.tile_pool(name="small", bufs=4) as small_pool,
            tc.tile_pool(name="xnb", bufs=2) as xnb_pool,
            tc.tile_pool(name="cm", bufs=2) as cm_pool,
            tc.tile_pool(name="acc", bufs=1) as acc_pool,
            tc.tile_pool(name="tp", bufs=2, space="PSUM") as tp_pool,
            tc.tile_pool(name="ps", bufs=3, space="PSUM") as ps_pool,
        ):
            negd = consts.tile([P, P], f32)
            nc.sync.dma_start(out=negd, in_=negdiag)
            idt = consts.tile([P, P], bf16)
            nc.sync.dma_start(out=idt, in_=identin)

            mbuf = acc_pool.tile([P, MCOLS], f32)
            # persistent transposed-normalized window [d-chunk k][cols]
            xnt = xnt_pool.tile([P, KC, COLS], dcomp)

            stats = {}

            def get_stats(t):
                if t not in stats:
                    n2 = stats_pool.tile([P, CH], f32, name=f"n2_{t}", tag="n2")
                    sc = stats_pool.tile([P, CH], f32, name=f"sc_{t}", tag="sc")
                    stats[t] = (n2, sc)
                return stats[t]

            def emit_prep_panel(tt, p):
                rep, t = divmod(tt, T)
                n2, sc = get_stats(tt)
                halves = []
                for half in range(2):
                    c0 = p * GRP + half * 4
                    xm = xin_pool.tile(
                        [P, 4, D], f32, name=f"xin_{tt}_{c0}", tag="xin"
                    )
                    nc.sync.dma_start(
                        out=xm,
                        in_=x[c0 * P:(c0 + 4) * P, t, :].rearrange(
                            "(c p) d -> p c d", p=P
                        ),
                    )
                    for j in range(4):
                        c = c0 + j
                        sqt = sq_pool.tile([P, D], bf16, name=f"sq_{tt}_{c}", tag="sq")
                        nc.scalar.activation(
                            out=sqt,
                            in_=xm[:, j, :],
                            func=AF.Square,
                            accum_out=n2[:, c:c + 1],
                        )
                    halves.append((c0, xm))

                # rsqrt seed via exp(-0.5 ln(n2)) (same ACT table set as
                # Square), then one fp32 Newton step:
                #   s = s0 * (1.5 - 0.5 * n2 * s0^2)
                gs = slice(p * GRP, (p + 1) * GRP)
                lnv = small_pool.tile([P, GRP], f32, name=f"lnv_{tt}_{p}", tag="lnv")
                nc.scalar.activation(out=lnv, in_=n2[:, gs], func=AF.Ln)
                s0 = small_pool.tile([P, GRP], f32, name=f"s0_{tt}_{p}", tag="s0")
                nc.scalar.activation(out=s0, in_=lnv, func=AF.Exp, scale=-0.5)
                t1 = small_pool.tile([P, GRP], f32, name=f"t1_{tt}_{p}", tag="t1")
                nc.vector.tensor_mul(t1, s0, s0)
                t2 = small_pool.tile([P, GRP], f32, name=f"t2_{tt}_{p}", tag="t2")
                nc.vector.tensor_mul(t2, t1, n2[:, gs])
                t3 = small_pool.tile([P, GRP], f32, name=f"t3_{tt}_{p}", tag="t3")
                nc.vector.tensor_scalar(t3, t2, -0.5, 1.5, ALU.mult, ALU.add)
                nc.vector.tensor_mul(sc[:, gs], s0, t3)

                for c0, xm in halves:
                    for j in range(4):
                        c = c0 + j
                        xnb = xnb_pool.tile(
                            [P, D], bf16, name=f"xnb_{tt}_{c}", tag="xnb"
                        )
                        nc.vector.tensor_scalar_mul(
                            xnb, xm[:, j, :], sc[:, c:c + 1]
                        )
                        tp = tp_pool.tile(
                            [P, KC, P], bf16, name=f"tp_{tt}_{c}", tag="tp"
                        )
                        for k in range(KC):
                            nc.tensor.transpose(
                                tp[:, k, :], xnb[:, k * P:(k + 1) * P], idt
                            )
                        nc.scalar.activation(
                            out=xnt[:, :, c * P:(c + 1) * P],
                            in_=tp,
                            func=AF.Copy,
                            scale=XS,
                        )

            def mi_base(mi):
                return (mi * P) // BW * BW

            def emit_gram_pg(tt, pg, qt, cm):
                """All (mi, pair-group pg) gram tiles. Per-mi windows are
                [mi_base, mi_base + WB*BW) so every unordered pair is
                covered by one side (d<4097 ahead per query row). Blocks
                are processed in QCW-wide pair-groups so one LDWEIGHTS
                serves two 512-col matmuls and DVE ops run 1024 wide."""
                rep, t = divmod(tt, T)
                # widest-window mi first: its tensor_copy initializes the
                # cm range that the narrower mis then max into
                mis = sorted(
                    range(MB),
                    key=lambda mi: -(
                        min(mi_base(mi) + WB * BW, (pg + 1) * QCW)
                        - max(mi_base(mi), pg * QCW)
                    ),
                )
                for i, mi in enumerate(mis):
                    base = mi_base(mi)
                    lo = max(base, pg * QCW)
                    hi = min(base + WB * BW, (pg + 1) * QCW)
                    W = hi - lo
                    assert W in (BW, QCW)
                    ps = ps_pool.tile(
                        [P, QCW], f32, name=f"ps_{tt}_{pg}_{mi}", tag="ps"
                    )
                    for nb in range(W // BW):
                        c0, c1 = lo + nb * BW, lo + (nb + 1) * BW
                        if fp8:
                            for kp in range(KC // 2):
                                nc.tensor.matmul(
                                    ps[:, nb * BW:(nb + 1) * BW],
                                    qt[:, 2 * kp:2 * kp + 2, mi * P:(mi + 1) * P],
                                    xnt[:, 2 * kp:2 * kp + 2, c0:c1],
                                    start=(kp == 0),
                                    stop=(kp == KC // 2 - 1),
                                    perf_mode=mybir.MatmulPerfMode.DoubleRow,
                                )
                        else:
                            for k in range(KC):
                                nc.tensor.matmul(
                                    ps[:, nb * BW:(nb + 1) * BW],
                                    qt[:, k, mi * P:(mi + 1) * P],
                                    xnt[:, k, c0:c1],
                                    start=(k == 0),
                                    stop=(k == KC - 1),
                                )
                    psw = ps[:, 0:W]
                    if pg == 0:
                        # mask the self-dot: psum diag window += -4*I
                        off = mi * P - lo
                        nc.vector.tensor_tensor(
                            ps[:, off:off + P],
                            ps[:, off:off + P],
                            negd,
                            op=ALU.add,
                        )
                    col = t * MB + mi
                    if pg == 0:
                        rm = mbuf[:, col:col + 1]
                    else:
                        rm = small_pool.tile(
                            [P, 1], f32, name=f"qm_{tt}_{pg}_{mi}", tag="qm"
                        )
                    nc.vector.reduce_max(rm, psw, axis=AX.X)
                    if i == 0:
                        nc.vector.tensor_copy(cm[:, lo:hi], psw)
                    else:
                        nc.vector.tensor_tensor(
                            cm[:, lo:hi],
                            cm[:, lo:hi],
                            psw,
                            op=ALU.max,
                        )
                    if pg != 0:
                        nc.vector.tensor_tensor(
                            mbuf[:, col:col + 1],
                            mbuf[:, col:col + 1],
                            rm,
                            op=ALU.max,
                        )

            # ---- emission schedule: pipeline prep(t+1) under gram(t) ----
            TT = repeat * T
            for p in range(NQW):
                emit_prep_panel(0, p)
            for tt in range(TT):
                t = tt % T
                qt = qt_pool.tile([P, KC, NQ], dcomp, name=f"qt_{tt}", tag="qt")
                nc.vector.tensor_copy(qt, xnt[:, :, 0:NQ])
                cm = cm_pool.tile([P, COLS], bf16, name=f"cm_{tt}", tag="cm")
                for pg in range(NQW):
                    emit_gram_pg(tt, pg, qt, cm)
                    if tt + 1 < TT:
                        emit_prep_panel(tt + 1, pg)
                nc.sync.dma_start(out=colmax[t, :, :], in_=cm)

            nc.sync.dma_start(out=maxes, in_=mbuf)

    nc.compile()
    return nc


def make_negdiag(maskval=None):
    if maskval is None:
        maskval = -4.0 * _XSCALE * _XSCALE
    return (maskval * np.eye(128)).astype(np.float32)


def make_ident():
    from concourse import mybir

    return np.eye(128).astype(mybir.dt.np(mybir.dt.bfloat16))


def make_in_maps(x, B=_B, T=_T, D=_D, ncores=_NCORES):
    """x: [B, T, D] fp32 full input -> per-core rolled input maps."""
    x = np.ascontiguousarray(x, dtype=np.float32)
    assert x.shape == (B, T, D)
    nd = make_negdiag()
    idt = make_ident()
    NQ = B // ncores
    in_maps = []
    for c in range(ncores):
        xr = np.roll(x, -c * NQ, axis=0) if c else x
        in_maps.append(
            {"x": np.ascontiguousarray(xr), "negdiag": nd, "ident": idt}
        )
    return in_maps


def combine_maxes(results, B=_B, T=_T, D=_D, ncores=_NCORES):
    """Combine per-core row/column max partials -> M [T, B] (fp64)."""
    P, NQ, MB, QCW, NQW, COLS, KC, CH, GRP, BW, NG, WB = _cfg(B, T, D, ncores)
    M = np.full((T, B), -np.inf)
    for c, r in enumerate(results):
        rowmax = np.asarray(r["maxes"], dtype=np.float64)  # [128, T*MB]
        for t in range(T):
            for mi in range(MB):
                rows = (c * NQ + mi * P + np.arange(P)) % B
                M[t, rows] = np.maximum(M[t, rows], rowmax[:, t * MB + mi])
        cmx = np.asarray(r["colmax"], dtype=np.float64)  # [T, 128, COLS]
        cmx = cmx.max(axis=1)  # [T, COLS]
        gcols = (c * NQ + np.arange(COLS)) % B
        for t in range(T):
            np.maximum.at(M[t], gcols, cmx[t])
    return M


def assemble_output(results, B=_B, T=_T, D=_D, ncores=_NCORES):
    M = combine_maxes(results, B, T, D, ncores) / (_XSCALE * _XSCALE)
    loss = -0.5 * np.log(2.0 - 2.0 * M).mean()
    return np.asarray(loss, dtype=np.float32)


def kernel(episodes_vectors: np.ndarray) -> np.ndarray:
    from concourse.bass_utils import run_bass_kernel_spmd

    key = (_B, _T, _D, _NCORES)
    if key not in _nc_cache:
        _nc_cache[key] = build_nc()
    nc = _nc_cache[key]

    in_maps = make_in_maps(episodes_vectors)
    last_err = None
    for _attempt in range(3):
        try:
            res = run_bass_kernel_spmd(nc, in_maps, list(range(_NCORES)))
            return assemble_output(res.results)
        except Exception as e:  # transient PJRT/tunnel INTERNAL errors
            last_err = e
    raise last_err


if __name__ == "__main__":
    inputs = {
        "episodes_vectors": np.random.default_rng(0)
        .standard_normal((_B, _T, _D))
        .astype(np.float32)
    }
    print(kernel(**inputs))
